# revision 7
# baseline (speedup 1.0000x reference)
"""MLA forward (DeepSeek-style) on 8 Trainium2 NeuronCores via Bass/Tile.

Two SPMD launches:
  A) down-projections, contraction-sharded over HIDDEN (8 x 896):
     each core computes a partial sum of [S, Q_LORA + ROPE + KV_LORA];
     host reduces the 8 partials.
  B) head-sharded (2 heads/core): RMSNorm'd low-rank inputs are
     up-projected, RoPE'd, attention + softmax run per head, attention
     probs written out, and the output projection accumulated over the
     core's 2 heads (host sums the 8 partial output projections).

All matmuls run in bf16 with fp32 PSUM accumulation; softmax is fp32.
"""

import sys

sys.path.insert(0, "/opt/trn_rl_repo")

import math

import numpy as np
import ml_dtypes

import concourse.bass as bass
import concourse.bacc as bacc
import concourse.mybir as mybir
from concourse.tile import TileContext
from concourse.masks import make_identity
from concourse.bass_utils import run_bass_kernel_spmd

BF16 = mybir.dt.bfloat16
F32 = mybir.dt.float32

HIDDEN = 7168
N_HEAD = 16
Q_LORA = 1536
KV_LORA = 512
ROPE_DIM = 64
NOPE_DIM = 128
V_DIM = 128
ROPE_BASE = 128000.0
EPS = 1e-6
QK_DIM = NOPE_DIM + ROPE_DIM  # 192
S = 2048
B = 1
NCORES = 8
HPC = N_HEAD // NCORES  # heads per core = 2
KSLICE = HIDDEN // NCORES  # 896
DOWN_COLS = Q_LORA + ROPE_DIM + KV_LORA  # 2112
INV_SQRT = 1.0 / math.sqrt(QK_DIM)
NEG = -1.0e30

_cache = {}


def _build_launch_a():
    nc = bacc.Bacc(None)
    h_in = nc.dram_tensor("h_in", [128, 7 * S], BF16, kind="ExternalInput")
    w_in = nc.dram_tensor("w_in", [128, 7 * DOWN_COLS], BF16, kind="ExternalInput")
    part = nc.dram_tensor("part", [S, DOWN_COLS], F32, kind="ExternalOutput")

    JT = [(0, 512), (512, 512), (1024, 512), (1536, 512), (2048, 64)]
    with TileContext(nc) as tc:
        with (
            tc.tile_pool(name="inp", bufs=1) as inp,
            tc.tile_pool(name="orow", bufs=3) as op,
            tc.tile_pool(name="ps", bufs=4, space="PSUM") as pp,
        ):
            hS = inp.tile([128, 7 * S], BF16)
            wS = inp.tile([128, 7 * DOWN_COLS], BF16)
            nc.sync.dma_start(hS[:, :], h_in[:, :])
            nc.sync.dma_start(wS[:, :], w_in[:, :])
            for st in range(16):
                orow = op.tile([128, DOWN_COLS], F32)
                for j0, jw in JT:
                    ps = pp.tile([128, 512], F32)
                    for kt in range(7):
                        nc.tensor.matmul(
                            ps[:, :jw],
                            hS[:, kt * S + st * 128 : kt * S + (st + 1) * 128],
                            wS[:, kt * DOWN_COLS + j0 : kt * DOWN_COLS + j0 + jw],
                            start=(kt == 0),
                            stop=(kt == 6),
                        )
                    nc.vector.tensor_copy(orow[:, j0 : j0 + jw], ps[:, :jw])
                nc.sync.dma_start(part[st * 128 : (st + 1) * 128, :], orow[:, :])
    nc.finalize()
    return nc


def _build_launch_b():
    nc = bacc.Bacc(None)
    qn_in = nc.dram_tensor("qn_in", [128, 12 * S], BF16, kind="ExternalInput")
    kv_in = nc.dram_tensor("kv_in", [128, 4 * S], BF16, kind="ExternalInput")
    kr_in = nc.dram_tensor("kr_in", [64, S], BF16, kind="ExternalInput")
    cos_in = nc.dram_tensor("cos_in", [64, S], F32, kind="ExternalInput")
    sin_in = nc.dram_tensor("sin_in", [64, S], F32, kind="ExternalInput")
    wq_in = nc.dram_tensor("wq_in", [128, 12 * 2 * QK_DIM], BF16, kind="ExternalInput")
    wkv_in = nc.dram_tensor("wkv_in", [128, 4 * 512], BF16, kind="ExternalInput")
    wo_in = nc.dram_tensor("wo_in", [128, 2 * HIDDEN], BF16, kind="ExternalInput")
    mask_in = nc.dram_tensor("mask_in", [128, 4 * 512], F32, kind="ExternalInput")
    attn_o = nc.dram_tensor("attn_o", [2 * S, S], F32, kind="ExternalOutput")
    out_o = nc.dram_tensor("out_o", [S, HIDDEN], BF16, kind="ExternalOutput")

    with TileContext(nc) as tc:
        with (
            tc.tile_pool(name="w", bufs=1) as wp,
            tc.tile_pool(name="act", bufs=1) as ap,
            tc.tile_pool(name="mm512", bufs=2, space="PSUM") as mmp,
            tc.tile_pool(name="s512", bufs=2, space="PSUM") as sp,
            tc.tile_pool(name="ptp", bufs=2, space="PSUM") as ptp,
            tc.tile_pool(name="aop", bufs=2, space="PSUM") as aop,
        ):
            wqS = wp.tile([128, 12 * 384], BF16)
            wkvS = wp.tile([128, 4 * 512], BF16)
            woS = wp.tile([128, 2 * HIDDEN], BF16)
            maskS = wp.tile([128, 4 * 512], F32)
            krS = wp.tile([64, S], BF16)
            cosS = wp.tile([64, S], F32)
            sinS = wp.tile([64, S], F32)
            ident = wp.tile([128, 128], BF16)
            nc.sync.dma_start(wqS[:, :], wq_in[:, :])
            nc.sync.dma_start(wkvS[:, :], wkv_in[:, :])
            nc.sync.dma_start(woS[:, :], wo_in[:, :])
            nc.sync.dma_start(maskS[:, :], mask_in[:, :])
            nc.sync.dma_start(krS[:, :], kr_in[:, :])
            nc.sync.dma_start(cosS[:, :], cos_in[:, :])
            nc.sync.dma_start(sinS[:, :], sin_in[:, :])
            make_identity(nc, ident[:, :])

            # per-head activations, resident
            qnopeT = [ap.tile([128, S], BF16, tag=f"qn{h}", name=f"qnopeT{h}") for h in range(2)]
            qropeT = [ap.tile([64, S], BF16, tag=f"qr{h}", name=f"qropeT{h}") for h in range(2)]
            knopeT = [ap.tile([128, S], BF16, tag=f"kn{h}", name=f"knopeT{h}") for h in range(2)]
            vS = [ap.tile([128, 16 * 128], BF16, tag=f"v{h}", name=f"vS{h}") for h in range(2)]
            aoT = [ap.tile([128, S], BF16, tag=f"ao{h}", name=f"aoT{h}") for h in range(2)]

            # ---- stage 1: q up-projection (+ RoPE on q) ----
            with (
                tc.tile_pool(name="qin", bufs=1) as qp,
                tc.tile_pool(name="rtmp", bufs=4) as rp,
            ):
                qnS = qp.tile([128, 12 * S], BF16)
                nc.sync.dma_start(qnS[:, :], qn_in[:, :])
                for h in range(2):
                    for st in range(4):
                        s0 = st * 512
                        # nope part: [128, 512] psum
                        ps = mmp.tile([128, 512], F32)
                        for kt in range(12):
                            nc.tensor.matmul(
                                ps[:, :],
                                wqS[:, kt * 384 + h * 192 : kt * 384 + h * 192 + 128],
                                qnS[:, kt * S + s0 : kt * S + s0 + 512],
                                start=(kt == 0),
                                stop=(kt == 11),
                            )
                        nc.vector.tensor_copy(qnopeT[h][:, s0 : s0 + 512], ps[:, :])
                        # rope part: [64, 512] psum, then rotate
                        pr = mmp.tile([128, 512], F32, tag="ps")
                        for kt in range(12):
                            nc.tensor.matmul(
                                pr[:64, :],
                                wqS[:, kt * 384 + h * 192 + 128 : kt * 384 + h * 192 + 192],
                                qnS[:, kt * S + s0 : kt * S + s0 + 512],
                                start=(kt == 0),
                                stop=(kt == 11),
                            )
                        tcos = rp.tile([64, 512], F32, tag="tcos")
                        trot = rp.tile([64, 512], F32, tag="trot")
                        nc.vector.tensor_mul(tcos[:, :], pr[:64, :], cosS[:, s0 : s0 + 512])
                        nc.vector.tensor_mul(trot[:32, :], pr[32:64, :], sinS[:32, s0 : s0 + 512])
                        nc.vector.tensor_mul(trot[32:64, :], pr[:32, :], sinS[32:64, s0 : s0 + 512])
                        nc.vector.tensor_sub(qropeT[h][:32, s0 : s0 + 512], tcos[:32, :], trot[:32, :])
                        nc.vector.tensor_add(qropeT[h][32:64, s0 : s0 + 512], tcos[32:64, :], trot[32:64, :])

            # ---- stage 2: k_nope / V up-projection ----
            with tc.tile_pool(name="kvin", bufs=1) as kp:
                kvS = kp.tile([128, 4 * S], BF16)
                nc.sync.dma_start(kvS[:, :], kv_in[:, :])
                for h in range(2):
                    for st in range(4):
                        s0 = st * 512
                        ps = mmp.tile([128, 512], F32)
                        for kt in range(4):
                            nc.tensor.matmul(
                                ps[:, :],
                                wkvS[:, kt * 512 + h * 256 : kt * 512 + h * 256 + 128],
                                kvS[:, kt * S + s0 : kt * S + s0 + 512],
                                start=(kt == 0),
                                stop=(kt == 3),
                            )
                        nc.vector.tensor_copy(knopeT[h][:, s0 : s0 + 512], ps[:, :])
                    for ks in range(16):
                        pv = aop.tile([128, 128], F32, tag="pa")
                        for kt in range(4):
                            nc.tensor.matmul(
                                pv[:, :],
                                kvS[:, kt * S + ks * 128 : kt * S + (ks + 1) * 128],
                                wkvS[:, kt * 512 + h * 256 + 128 : kt * 512 + h * 256 + 256],
                                start=(kt == 0),
                                stop=(kt == 3),
                            )
                        nc.vector.tensor_copy(vS[h][:, ks * 128 : (ks + 1) * 128], pv[:, :])

            # ---- stage 3: attention per head ----
            with (
                tc.tile_pool(name="srow", bufs=2) as srp,
                tc.tile_pool(name="p32", bufs=2) as p32p,
                tc.tile_pool(name="af", bufs=2) as afp,
                tc.tile_pool(name="pb", bufs=2) as pbp,
                tc.tile_pool(name="pt", bufs=3) as pts,
                tc.tile_pool(name="stat", bufs=8) as stp,
            ):
                for h in range(2):
                    for qt in range(16):
                        dt = qt // 4
                        kend = 512 * (dt + 1)
                        q0 = qt * 128
                        srow = srp.tile([128, S], F32)
                        for kt in range(dt + 1):
                            k0 = kt * 512
                            ps = sp.tile([128, 512], F32)
                            nc.tensor.matmul(
                                ps[:, :],
                                qnopeT[h][:, q0 : q0 + 128],
                                knopeT[h][:, k0 : k0 + 512],
                                start=True,
                                stop=False,
                            )
                            nc.tensor.matmul(
                                ps[:, :],
                                qropeT[h][:, q0 : q0 + 128],
                                krS[:, k0 : k0 + 512],
                                start=False,
                                stop=True,
                            )
                            if kt == dt:
                                m = qt % 4
                                nc.vector.tensor_add(
                                    srow[:, k0 : k0 + 512],
                                    ps[:, :],
                                    maskS[:, m * 512 : (m + 1) * 512],
                                )
                            else:
                                nc.vector.tensor_copy(srow[:, k0 : k0 + 512], ps[:, :])
                        mx = stp.tile([128, 1], F32, tag="mx")
                        nb = stp.tile([128, 1], F32, tag="nb")
                        sm = stp.tile([128, 1], F32, tag="sm")
                        rr = stp.tile([128, 1], F32, tag="rr")
                        nc.vector.reduce_max(mx[:, :], srow[:, :kend], axis=mybir.AxisListType.X)
                        nc.vector.tensor_scalar_mul(nb[:, :], mx[:, :], -INV_SQRT)
                        p32 = p32p.tile([128, S], F32)
                        nc.scalar.activation(
                            p32[:, :kend],
                            srow[:, :kend],
                            mybir.ActivationFunctionType.Exp,
                            bias=nb[:, :],
                            scale=INV_SQRT,
                            accum_out=sm[:, :],
                        )
                        nc.vector.reciprocal(rr[:, :], sm[:, :])
                        af = afp.tile([128, S], F32)
                        nc.vector.tensor_scalar_mul(af[:, :kend], p32[:, :kend], rr[:, :])
                        nc.sync.dma_start(
                            attn_o[h * S + q0 : h * S + q0 + 128, 0:kend], af[:, :kend]
                        )
                        pb = pbp.tile([128, S], BF16)
                        nc.scalar.mul(pb[:, :kend], p32[:, :kend], rr[:, :])
                        # transpose P and accumulate A @ V (transposed): aoT = V^T P^T
                        pa = aop.tile([128, 128], F32, tag="pa")
                        for kb in range(qt + 1):
                            pt_ps = ptp.tile([128, 128], BF16)
                            nc.tensor.transpose(
                                pt_ps[:, :], pb[:, kb * 128 : (kb + 1) * 128], ident[:, :]
                            )
                            ptile = pts.tile([128, 128], BF16)
                            nc.scalar.copy(ptile[:, :], pt_ps[:, :])
                            nc.tensor.matmul(
                                pa[:, :],
                                vS[h][:, kb * 128 : (kb + 1) * 128],
                                ptile[:, :],
                                start=(kb == 0),
                                stop=(kb == qt),
                            )
                        nc.vector.tensor_copy(aoT[h][:, q0 : q0 + 128], pa[:, :])

            # ---- stage 4: output projection (partial over this core's heads) ----
            with tc.tile_pool(name="orow", bufs=2) as orp:
                for st in range(16):
                    s0 = st * 128
                    orow = orp.tile([128, HIDDEN], BF16)
                    for jt in range(14):
                        j0 = jt * 512
                        ps = mmp.tile([128, 512], F32)
                        nc.tensor.matmul(
                            ps[:, :],
                            aoT[0][:, s0 : s0 + 128],
                            woS[:, j0 : j0 + 512],
                            start=True,
                            stop=False,
                        )
                        nc.tensor.matmul(
                            ps[:, :],
                            aoT[1][:, s0 : s0 + 128],
                            woS[:, HIDDEN + j0 : HIDDEN + j0 + 512],
                            start=False,
                            stop=True,
                        )
                        nc.vector.tensor_copy(orow[:, j0 : j0 + 512], ps[:, :])
                    nc.sync.dma_start(out_o[s0 : s0 + 128, :], orow[:, :])
    nc.finalize()
    return nc


def _get_progs():
    if "A" not in _cache:
        _cache["A"] = _build_launch_a()
        _cache["B"] = _build_launch_b()
    return _cache["A"], _cache["B"]


def _pack_p_major(arr, nt, width):
    # [nt*128, width] -> [128, nt*width] with [p, t*width + j] = arr[t*128+p, j]
    return np.ascontiguousarray(
        arr.reshape(nt, 128, width).swapaxes(0, 1).reshape(128, nt * width)
    )


def _to_bf16(a):
    return np.ascontiguousarray(a.astype(ml_dtypes.bfloat16))


def kernel(hidden_states, position_ids, Wq_down, q_norm_w, Wq_up,
           Wkv_down, kv_norm_w, Wkv_up, Wout, _trace=False):
    nc_a, nc_b = _get_progs()
    h = np.asarray(hidden_states, np.float32)[0]  # [S, HIDDEN]
    pos = np.asarray(position_ids).reshape(-1).astype(np.int64)

    # ---- launch A: down projections, contraction-sharded ----
    hT = _to_bf16(h.T)  # [HIDDEN, S]
    Wd = _to_bf16(np.concatenate([np.asarray(Wq_down, np.float32),
                                  np.asarray(Wkv_down, np.float32)], axis=1))
    in_maps_a = []
    for c in range(NCORES):
        r0 = c * KSLICE
        in_maps_a.append({
            "h_in": _pack_p_major(hT[r0 : r0 + KSLICE], 7, S),
            "w_in": _pack_p_major(Wd[r0 : r0 + KSLICE], 7, DOWN_COLS),
        })
    res_a = run_bass_kernel_spmd(nc_a, in_maps_a, core_ids=list(range(NCORES)),
                                 trace=_trace)
    acc = np.zeros((S, DOWN_COLS), np.float32)
    for c in range(NCORES):
        acc += res_a.results[c]["part"]

    qd = acc[:, :Q_LORA]
    kr_raw = acc[:, Q_LORA : Q_LORA + ROPE_DIM]
    kvh = acc[:, Q_LORA + ROPE_DIM :]

    # ---- host glue: RMSNorm, RoPE tables, k RoPE, repacks ----
    qrms = np.sqrt(np.mean(qd * qd, axis=-1, keepdims=True))
    qn = np.asarray(q_norm_w, np.float32) * (qd / (qrms + EPS))
    krms = np.sqrt(np.mean(kvh * kvh, axis=-1, keepdims=True))
    kvn = np.asarray(kv_norm_w, np.float32) * (kvh / (krms + EPS))

    inv_freq = 1.0 / (ROPE_BASE ** (np.arange(0, ROPE_DIM, 2, dtype=np.float32) / ROPE_DIM))
    t = np.arange(S, dtype=np.float32)
    freqs = np.outer(t, inv_freq)
    emb = np.concatenate([freqs, freqs], axis=-1)  # [S, 64]
    cos = np.cos(emb)[pos]  # [S, 64]
    sin = np.sin(emb)[pos]

    # interleaved -> half reorder, then rope, for the shared k_rope head
    kr_p = np.empty_like(kr_raw)
    kr_p[:, : ROPE_DIM // 2] = kr_raw[:, 0::2]
    kr_p[:, ROPE_DIM // 2 :] = kr_raw[:, 1::2]
    rot = np.concatenate([-kr_p[:, ROPE_DIM // 2 :], kr_p[:, : ROPE_DIM // 2]], axis=1)
    kr = kr_p * cos + rot * sin

    qn_in = _pack_p_major(_to_bf16(qn.T), 12, S)
    kv_in = _pack_p_major(_to_bf16(kvn.T), 4, S)
    kr_in = _to_bf16(kr.T)
    cos_in = np.ascontiguousarray(cos.T)
    sin_in = np.ascontiguousarray(sin.T)

    # weights with the q-rope interleave folded into Wq_up columns
    Wq_up_r = np.asarray(Wq_up, np.float32).reshape(Q_LORA, N_HEAD, QK_DIM)
    Wq_perm = Wq_up_r.copy()
    Wq_perm[:, :, NOPE_DIM : NOPE_DIM + ROPE_DIM // 2] = Wq_up_r[:, :, NOPE_DIM::2]
    Wq_perm[:, :, NOPE_DIM + ROPE_DIM // 2 :] = Wq_up_r[:, :, NOPE_DIM + 1 :: 2]
    Wkv_up_r = np.asarray(Wkv_up, np.float32).reshape(KV_LORA, N_HEAD, NOPE_DIM + V_DIM)
    Wout_f = np.asarray(Wout, np.float32)

    mask = np.full((128, 4, 512), NEG, np.float32)
    for m in range(4):
        for p in range(128):
            mask[p, m, : 128 * m + p + 1] = 0.0
    mask_in = np.ascontiguousarray(mask.reshape(128, 4 * 512))

    in_maps_b = []
    for c in range(NCORES):
        wq_c = _to_bf16(Wq_perm[:, 2 * c : 2 * c + 2, :].reshape(Q_LORA, 2 * QK_DIM))
        wkv_c = _to_bf16(Wkv_up_r[:, 2 * c : 2 * c + 2, :].reshape(KV_LORA, 512))
        wo_c = _to_bf16(Wout_f[256 * c : 256 * (c + 1)])
        in_maps_b.append({
            "qn_in": qn_in,
            "kv_in": kv_in,
            "kr_in": kr_in,
            "cos_in": cos_in,
            "sin_in": sin_in,
            "wq_in": _pack_p_major(wq_c, 12, 2 * QK_DIM),
            "wkv_in": _pack_p_major(wkv_c, 4, 512),
            "wo_in": _pack_p_major(wo_c, 2, HIDDEN),
            "mask_in": mask_in,
        })
    res_b = run_bass_kernel_spmd(nc_b, in_maps_b, core_ids=list(range(NCORES)),
                                 trace=_trace)

    attn = np.empty((1, N_HEAD, S, S), np.float32)
    out = np.zeros((S, HIDDEN), np.float32)
    for c in range(NCORES):
        attn[0, 2 * c : 2 * c + 2] = res_b.results[c]["attn_o"].reshape(2, S, S)
        out += res_b.results[c]["out_o"].astype(np.float32)

    if _trace:
        kernel._last_exec_ns = (res_a.exec_time_ns or 0) + (res_b.exec_time_ns or 0)
        kernel._last_parts = (res_a.exec_time_ns, res_b.exec_time_ns)
    return out.reshape(B, S, HIDDEN), attn


# revision 8
# speedup vs baseline: 1.0243x; 1.0243x over previous
"""MLA forward (DeepSeek-style) on 8 Trainium2 NeuronCores via Bass/Tile.

Two SPMD launches:
  A) down-projections, contraction-sharded over HIDDEN (8 x 896):
     each core computes a partial sum of [S, Q_LORA + ROPE + KV_LORA];
     host reduces the 8 partials.
  B) head-sharded (2 heads/core): RMSNorm'd low-rank inputs are
     up-projected, RoPE'd, attention + softmax run per head, attention
     probs written out, and the output projection accumulated over the
     core's 2 heads (host sums the 8 partial output projections).

All matmuls run in bf16 with fp32 PSUM accumulation; softmax is fp32.
"""

import sys

sys.path.insert(0, "/opt/trn_rl_repo")

import math

import numpy as np
import ml_dtypes

import concourse.bass as bass
import concourse.bacc as bacc
import concourse.mybir as mybir
from concourse.tile import TileContext
from concourse.masks import make_identity
from concourse.bass_utils import run_bass_kernel_spmd

BF16 = mybir.dt.bfloat16
F32 = mybir.dt.float32

HIDDEN = 7168
N_HEAD = 16
Q_LORA = 1536
KV_LORA = 512
ROPE_DIM = 64
NOPE_DIM = 128
V_DIM = 128
ROPE_BASE = 128000.0
EPS = 1e-6
QK_DIM = NOPE_DIM + ROPE_DIM  # 192
S = 2048
B = 1
NCORES = 8
HPC = N_HEAD // NCORES  # heads per core = 2
KSLICE = HIDDEN // NCORES  # 896
DOWN_COLS = Q_LORA + ROPE_DIM + KV_LORA  # 2112
INV_SQRT = 1.0 / math.sqrt(QK_DIM)
NEG = -1.0e30

_cache = {}


def _build_launch_a():
    nc = bacc.Bacc(None)
    h_in = nc.dram_tensor("h_in", [128, 7 * S], BF16, kind="ExternalInput")
    w_in = nc.dram_tensor("w_in", [128, 7 * DOWN_COLS], BF16, kind="ExternalInput")
    part = nc.dram_tensor("part", [S, DOWN_COLS], F32, kind="ExternalOutput")

    JT = [(0, 512), (512, 512), (1024, 512), (1536, 512), (2048, 64)]
    with TileContext(nc) as tc:
        with (
            tc.tile_pool(name="inp", bufs=1) as inp,
            tc.tile_pool(name="orow", bufs=3) as op,
            tc.tile_pool(name="ps", bufs=4, space="PSUM") as pp,
        ):
            hS = inp.tile([128, 7 * S], BF16)
            wS = inp.tile([128, 7 * DOWN_COLS], BF16)
            nc.sync.dma_start(hS[:, :], h_in[:, :])
            nc.sync.dma_start(wS[:, :], w_in[:, :])
            for st in range(16):
                orow = op.tile([128, DOWN_COLS], F32)
                for j0, jw in JT:
                    ps = pp.tile([128, 512], F32)
                    for kt in range(7):
                        nc.tensor.matmul(
                            ps[:, :jw],
                            hS[:, kt * S + st * 128 : kt * S + (st + 1) * 128],
                            wS[:, kt * DOWN_COLS + j0 : kt * DOWN_COLS + j0 + jw],
                            start=(kt == 0),
                            stop=(kt == 6),
                        )
                    nc.vector.tensor_copy(orow[:, j0 : j0 + jw], ps[:, :jw])
                nc.sync.dma_start(part[st * 128 : (st + 1) * 128, :], orow[:, :])
    nc.finalize()
    return nc


def _build_launch_b():
    nc = bacc.Bacc(None)
    qn_in = nc.dram_tensor("qn_in", [128, 12 * S], BF16, kind="ExternalInput")
    kv_in = nc.dram_tensor("kv_in", [128, 4 * S], BF16, kind="ExternalInput")
    kr_in = nc.dram_tensor("kr_in", [64, S], BF16, kind="ExternalInput")
    cos_in = nc.dram_tensor("cos_in", [64, S], F32, kind="ExternalInput")
    sin_in = nc.dram_tensor("sin_in", [64, S], F32, kind="ExternalInput")
    wq_in = nc.dram_tensor("wq_in", [128, 12 * 2 * QK_DIM], BF16, kind="ExternalInput")
    wkv_in = nc.dram_tensor("wkv_in", [128, 4 * 512], BF16, kind="ExternalInput")
    wo_in = nc.dram_tensor("wo_in", [128, 2 * HIDDEN], BF16, kind="ExternalInput")
    mask_in = nc.dram_tensor("mask_in", [128, 4 * 512], F32, kind="ExternalInput")
    attn_o = nc.dram_tensor("attn_o", [2 * S, S], F32, kind="ExternalOutput")
    out_o = nc.dram_tensor("out_o", [S, HIDDEN], BF16, kind="ExternalOutput")

    with TileContext(nc) as tc:
        with (
            tc.tile_pool(name="w", bufs=1) as wp,
            tc.tile_pool(name="act", bufs=1) as ap,
            tc.tile_pool(name="mm512", bufs=2, space="PSUM") as mmp,
            tc.tile_pool(name="s512", bufs=2, space="PSUM") as sp,
            tc.tile_pool(name="ptp", bufs=2, space="PSUM") as ptp,
            tc.tile_pool(name="aop", bufs=2, space="PSUM") as aop,
        ):
            wqS = wp.tile([128, 12 * 384], BF16)
            wkvS = wp.tile([128, 4 * 512], BF16)
            woS = wp.tile([128, 2 * HIDDEN], BF16)
            maskS = wp.tile([128, 4 * 512], F32)
            krS = wp.tile([64, S], BF16)
            cosS = wp.tile([64, S], F32)
            sinS = wp.tile([64, S], F32)
            ident = wp.tile([128, 128], BF16)
            nc.sync.dma_start(wqS[:, :], wq_in[:, :])
            nc.sync.dma_start(wkvS[:, :], wkv_in[:, :])
            nc.sync.dma_start(woS[:, :], wo_in[:, :])
            nc.sync.dma_start(maskS[:, :], mask_in[:, :])
            nc.sync.dma_start(krS[:, :], kr_in[:, :])
            nc.sync.dma_start(cosS[:, :], cos_in[:, :])
            nc.sync.dma_start(sinS[:, :], sin_in[:, :])
            make_identity(nc, ident[:, :])

            # per-head activations, resident
            qnopeT = [ap.tile([128, S], BF16, tag=f"qn{h}", name=f"qnopeT{h}") for h in range(2)]
            qropeT = [ap.tile([64, S], BF16, tag=f"qr{h}", name=f"qropeT{h}") for h in range(2)]
            knopeT = [ap.tile([128, S], BF16, tag=f"kn{h}", name=f"knopeT{h}") for h in range(2)]
            vS = [ap.tile([128, 16 * 128], BF16, tag=f"v{h}", name=f"vS{h}") for h in range(2)]
            aoT = [ap.tile([128, S], BF16, tag=f"ao{h}", name=f"aoT{h}") for h in range(2)]

            # ---- stage 1: q up-projection (+ RoPE on q) ----
            with (
                tc.tile_pool(name="qin", bufs=1) as qp,
                tc.tile_pool(name="rtmp", bufs=4) as rp,
            ):
                qnS = qp.tile([128, 12 * S], BF16)
                nc.sync.dma_start(qnS[:, :], qn_in[:, :])
                for h in range(2):
                    for st in range(4):
                        s0 = st * 512
                        # nope part: [128, 512] psum
                        ps = mmp.tile([128, 512], F32)
                        for kt in range(12):
                            nc.tensor.matmul(
                                ps[:, :],
                                wqS[:, kt * 384 + h * 192 : kt * 384 + h * 192 + 128],
                                qnS[:, kt * S + s0 : kt * S + s0 + 512],
                                start=(kt == 0),
                                stop=(kt == 11),
                            )
                        if st % 2 == 0:
                            nc.vector.tensor_copy(qnopeT[h][:, s0 : s0 + 512], ps[:, :])
                        else:
                            nc.scalar.copy(qnopeT[h][:, s0 : s0 + 512], ps[:, :])
                        # rope part: [64, 512] psum, then rotate
                        pr = mmp.tile([128, 512], F32, tag="ps")
                        for kt in range(12):
                            nc.tensor.matmul(
                                pr[:64, :],
                                wqS[:, kt * 384 + h * 192 + 128 : kt * 384 + h * 192 + 192],
                                qnS[:, kt * S + s0 : kt * S + s0 + 512],
                                start=(kt == 0),
                                stop=(kt == 11),
                            )
                        tcos = rp.tile([64, 512], F32, tag="tcos")
                        trot = rp.tile([64, 512], F32, tag="trot")
                        nc.vector.tensor_mul(tcos[:, :], pr[:64, :], cosS[:, s0 : s0 + 512])
                        nc.vector.tensor_mul(trot[:32, :], pr[32:64, :], sinS[:32, s0 : s0 + 512])
                        nc.vector.tensor_mul(trot[32:64, :], pr[:32, :], sinS[32:64, s0 : s0 + 512])
                        nc.vector.tensor_sub(qropeT[h][:32, s0 : s0 + 512], tcos[:32, :], trot[:32, :])
                        nc.vector.tensor_add(qropeT[h][32:64, s0 : s0 + 512], tcos[32:64, :], trot[32:64, :])

            # ---- stage 2: k_nope / V up-projection ----
            with tc.tile_pool(name="kvin", bufs=1) as kp:
                kvS = kp.tile([128, 4 * S], BF16)
                nc.sync.dma_start(kvS[:, :], kv_in[:, :])
                for h in range(2):
                    for st in range(4):
                        s0 = st * 512
                        ps = mmp.tile([128, 512], F32)
                        for kt in range(4):
                            nc.tensor.matmul(
                                ps[:, :],
                                wkvS[:, kt * 512 + h * 256 : kt * 512 + h * 256 + 128],
                                kvS[:, kt * S + s0 : kt * S + s0 + 512],
                                start=(kt == 0),
                                stop=(kt == 3),
                            )
                        nc.vector.tensor_copy(knopeT[h][:, s0 : s0 + 512], ps[:, :])
                    for ks in range(16):
                        pv = aop.tile([128, 128], F32, tag="pa")
                        for kt in range(4):
                            nc.tensor.matmul(
                                pv[:, :],
                                kvS[:, kt * S + ks * 128 : kt * S + (ks + 1) * 128],
                                wkvS[:, kt * 512 + h * 256 + 128 : kt * 512 + h * 256 + 256],
                                start=(kt == 0),
                                stop=(kt == 3),
                            )
                        nc.vector.tensor_copy(vS[h][:, ks * 128 : (ks + 1) * 128], pv[:, :])

            # ---- stage 3: attention per head ----
            with (
                tc.tile_pool(name="srow", bufs=2) as srp,
                tc.tile_pool(name="p32", bufs=2) as p32p,
                tc.tile_pool(name="af", bufs=2) as afp,
                tc.tile_pool(name="pb", bufs=2) as pbp,
                tc.tile_pool(name="pt", bufs=3) as pts,
                tc.tile_pool(name="stat", bufs=8) as stp,
            ):
                for h in range(2):
                    for qt in range(16):
                        dt = qt // 4
                        kend = 512 * (dt + 1)
                        q0 = qt * 128
                        srow = srp.tile([128, S], F32)
                        for kt in range(dt + 1):
                            k0 = kt * 512
                            ps = sp.tile([128, 512], F32)
                            nc.tensor.matmul(
                                ps[:, :],
                                qnopeT[h][:, q0 : q0 + 128],
                                knopeT[h][:, k0 : k0 + 512],
                                start=True,
                                stop=False,
                            )
                            nc.tensor.matmul(
                                ps[:, :],
                                qropeT[h][:, q0 : q0 + 128],
                                krS[:, k0 : k0 + 512],
                                start=False,
                                stop=True,
                            )
                            if kt == dt:
                                m = qt % 4
                                nc.vector.tensor_add(
                                    srow[:, k0 : k0 + 512],
                                    ps[:, :],
                                    maskS[:, m * 512 : (m + 1) * 512],
                                )
                            else:
                                nc.vector.tensor_copy(srow[:, k0 : k0 + 512], ps[:, :])
                        mx = stp.tile([128, 1], F32, tag="mx")
                        nb = stp.tile([128, 1], F32, tag="nb")
                        sm = stp.tile([128, 1], F32, tag="sm")
                        rr = stp.tile([128, 1], F32, tag="rr")
                        nc.vector.reduce_max(mx[:, :], srow[:, :kend], axis=mybir.AxisListType.X)
                        nc.vector.tensor_scalar_mul(nb[:, :], mx[:, :], -INV_SQRT)
                        p32 = p32p.tile([128, S], F32)
                        nc.scalar.activation(
                            p32[:, :kend],
                            srow[:, :kend],
                            mybir.ActivationFunctionType.Exp,
                            bias=nb[:, :],
                            scale=INV_SQRT,
                            accum_out=sm[:, :],
                        )
                        nc.vector.reciprocal(rr[:, :], sm[:, :])
                        af = afp.tile([128, S], F32)
                        nc.vector.tensor_scalar_mul(af[:, :kend], p32[:, :kend], rr[:, :])
                        nc.sync.dma_start(
                            attn_o[h * S + q0 : h * S + q0 + 128, 0:kend], af[:, :kend]
                        )
                        pb = pbp.tile([128, S], BF16)
                        nc.scalar.mul(pb[:, :kend], p32[:, :kend], rr[:, :])
                        # transpose P and accumulate A @ V (transposed): aoT = V^T P^T
                        pa = aop.tile([128, 128], F32, tag="pa")
                        for kb in range(qt + 1):
                            pt_ps = ptp.tile([128, 128], BF16)
                            nc.tensor.transpose(
                                pt_ps[:, :], pb[:, kb * 128 : (kb + 1) * 128], ident[:, :]
                            )
                            ptile = pts.tile([128, 128], BF16)
                            nc.scalar.copy(ptile[:, :], pt_ps[:, :])
                            nc.tensor.matmul(
                                pa[:, :],
                                vS[h][:, kb * 128 : (kb + 1) * 128],
                                ptile[:, :],
                                start=(kb == 0),
                                stop=(kb == qt),
                            )
                        nc.vector.tensor_copy(aoT[h][:, q0 : q0 + 128], pa[:, :])

            # ---- stage 4: output projection (partial over this core's heads) ----
            with tc.tile_pool(name="orow", bufs=2) as orp:
                for st in range(16):
                    s0 = st * 128
                    orow = orp.tile([128, HIDDEN], BF16)
                    for jt in range(14):
                        j0 = jt * 512
                        ps = mmp.tile([128, 512], F32)
                        nc.tensor.matmul(
                            ps[:, :],
                            aoT[0][:, s0 : s0 + 128],
                            woS[:, j0 : j0 + 512],
                            start=True,
                            stop=False,
                        )
                        nc.tensor.matmul(
                            ps[:, :],
                            aoT[1][:, s0 : s0 + 128],
                            woS[:, HIDDEN + j0 : HIDDEN + j0 + 512],
                            start=False,
                            stop=True,
                        )
                        if jt % 2 == 0:
                            nc.vector.tensor_copy(orow[:, j0 : j0 + 512], ps[:, :])
                        else:
                            nc.scalar.copy(orow[:, j0 : j0 + 512], ps[:, :])
                    nc.sync.dma_start(out_o[s0 : s0 + 128, :], orow[:, :])
    nc.finalize()
    return nc


def _get_progs():
    if "A" not in _cache:
        _cache["A"] = _build_launch_a()
        _cache["B"] = _build_launch_b()
    return _cache["A"], _cache["B"]


def _pack_p_major(arr, nt, width):
    # [nt*128, width] -> [128, nt*width] with [p, t*width + j] = arr[t*128+p, j]
    return np.ascontiguousarray(
        arr.reshape(nt, 128, width).swapaxes(0, 1).reshape(128, nt * width)
    )


def _to_bf16(a):
    return np.ascontiguousarray(a.astype(ml_dtypes.bfloat16))


def kernel(hidden_states, position_ids, Wq_down, q_norm_w, Wq_up,
           Wkv_down, kv_norm_w, Wkv_up, Wout, _trace=False):
    nc_a, nc_b = _get_progs()
    h = np.asarray(hidden_states, np.float32)[0]  # [S, HIDDEN]
    pos = np.asarray(position_ids).reshape(-1).astype(np.int64)

    # ---- launch A: down projections, contraction-sharded ----
    hT = _to_bf16(h.T)  # [HIDDEN, S]
    Wd = _to_bf16(np.concatenate([np.asarray(Wq_down, np.float32),
                                  np.asarray(Wkv_down, np.float32)], axis=1))
    in_maps_a = []
    for c in range(NCORES):
        r0 = c * KSLICE
        in_maps_a.append({
            "h_in": _pack_p_major(hT[r0 : r0 + KSLICE], 7, S),
            "w_in": _pack_p_major(Wd[r0 : r0 + KSLICE], 7, DOWN_COLS),
        })
    res_a = run_bass_kernel_spmd(nc_a, in_maps_a, core_ids=list(range(NCORES)),
                                 trace=_trace)
    acc = np.zeros((S, DOWN_COLS), np.float32)
    for c in range(NCORES):
        acc += res_a.results[c]["part"]

    qd = acc[:, :Q_LORA]
    kr_raw = acc[:, Q_LORA : Q_LORA + ROPE_DIM]
    kvh = acc[:, Q_LORA + ROPE_DIM :]

    # ---- host glue: RMSNorm, RoPE tables, k RoPE, repacks ----
    qrms = np.sqrt(np.mean(qd * qd, axis=-1, keepdims=True))
    qn = np.asarray(q_norm_w, np.float32) * (qd / (qrms + EPS))
    krms = np.sqrt(np.mean(kvh * kvh, axis=-1, keepdims=True))
    kvn = np.asarray(kv_norm_w, np.float32) * (kvh / (krms + EPS))

    inv_freq = 1.0 / (ROPE_BASE ** (np.arange(0, ROPE_DIM, 2, dtype=np.float32) / ROPE_DIM))
    t = np.arange(S, dtype=np.float32)
    freqs = np.outer(t, inv_freq)
    emb = np.concatenate([freqs, freqs], axis=-1)  # [S, 64]
    cos = np.cos(emb)[pos]  # [S, 64]
    sin = np.sin(emb)[pos]

    # interleaved -> half reorder, then rope, for the shared k_rope head
    kr_p = np.empty_like(kr_raw)
    kr_p[:, : ROPE_DIM // 2] = kr_raw[:, 0::2]
    kr_p[:, ROPE_DIM // 2 :] = kr_raw[:, 1::2]
    rot = np.concatenate([-kr_p[:, ROPE_DIM // 2 :], kr_p[:, : ROPE_DIM // 2]], axis=1)
    kr = kr_p * cos + rot * sin

    qn_in = _pack_p_major(_to_bf16(qn.T), 12, S)
    kv_in = _pack_p_major(_to_bf16(kvn.T), 4, S)
    kr_in = _to_bf16(kr.T)
    cos_in = np.ascontiguousarray(cos.T)
    sin_in = np.ascontiguousarray(sin.T)

    # weights with the q-rope interleave folded into Wq_up columns
    Wq_up_r = np.asarray(Wq_up, np.float32).reshape(Q_LORA, N_HEAD, QK_DIM)
    Wq_perm = Wq_up_r.copy()
    Wq_perm[:, :, NOPE_DIM : NOPE_DIM + ROPE_DIM // 2] = Wq_up_r[:, :, NOPE_DIM::2]
    Wq_perm[:, :, NOPE_DIM + ROPE_DIM // 2 :] = Wq_up_r[:, :, NOPE_DIM + 1 :: 2]
    Wkv_up_r = np.asarray(Wkv_up, np.float32).reshape(KV_LORA, N_HEAD, NOPE_DIM + V_DIM)
    Wout_f = np.asarray(Wout, np.float32)

    mask = np.full((128, 4, 512), NEG, np.float32)
    for m in range(4):
        for p in range(128):
            mask[p, m, : 128 * m + p + 1] = 0.0
    mask_in = np.ascontiguousarray(mask.reshape(128, 4 * 512))

    in_maps_b = []
    for c in range(NCORES):
        wq_c = _to_bf16(Wq_perm[:, 2 * c : 2 * c + 2, :].reshape(Q_LORA, 2 * QK_DIM))
        wkv_c = _to_bf16(Wkv_up_r[:, 2 * c : 2 * c + 2, :].reshape(KV_LORA, 512))
        wo_c = _to_bf16(Wout_f[256 * c : 256 * (c + 1)])
        in_maps_b.append({
            "qn_in": qn_in,
            "kv_in": kv_in,
            "kr_in": kr_in,
            "cos_in": cos_in,
            "sin_in": sin_in,
            "wq_in": _pack_p_major(wq_c, 12, 2 * QK_DIM),
            "wkv_in": _pack_p_major(wkv_c, 4, 512),
            "wo_in": _pack_p_major(wo_c, 2, HIDDEN),
            "mask_in": mask_in,
        })
    res_b = run_bass_kernel_spmd(nc_b, in_maps_b, core_ids=list(range(NCORES)),
                                 trace=_trace)

    attn = np.empty((1, N_HEAD, S, S), np.float32)
    out = np.zeros((S, HIDDEN), np.float32)
    for c in range(NCORES):
        attn[0, 2 * c : 2 * c + 2] = res_b.results[c]["attn_o"].reshape(2, S, S)
        out += res_b.results[c]["out_o"].astype(np.float32)

    if _trace:
        kernel._last_exec_ns = (res_a.exec_time_ns or 0) + (res_b.exec_time_ns or 0)
        kernel._last_parts = (res_a.exec_time_ns, res_b.exec_time_ns)
    return out.reshape(B, S, HIDDEN), attn


# revision 9
# speedup vs baseline: 1.0591x; 1.0340x over previous
"""MLA forward (DeepSeek-style) on 8 Trainium2 NeuronCores via Bass/Tile.

Two SPMD launches:
  A) down-projections, contraction-sharded over HIDDEN (8 x 896):
     each core computes a partial sum of [S, Q_LORA + ROPE + KV_LORA];
     host reduces the 8 partials.
  B) head-sharded (2 heads/core): RMSNorm'd low-rank inputs are
     up-projected, RoPE'd, attention + softmax run per head, attention
     probs written out, and the output projection accumulated over the
     core's 2 heads (host sums the 8 partial output projections).

All matmuls run in bf16 with fp32 PSUM accumulation; softmax is fp32.
"""

import sys

sys.path.insert(0, "/opt/trn_rl_repo")

import math

import numpy as np
import ml_dtypes

import concourse.bass as bass
import concourse.bacc as bacc
import concourse.mybir as mybir
from concourse.tile import TileContext
from concourse.masks import make_identity
from concourse.bass_utils import run_bass_kernel_spmd

BF16 = mybir.dt.bfloat16
F32 = mybir.dt.float32

HIDDEN = 7168
N_HEAD = 16
Q_LORA = 1536
KV_LORA = 512
ROPE_DIM = 64
NOPE_DIM = 128
V_DIM = 128
ROPE_BASE = 128000.0
EPS = 1e-6
QK_DIM = NOPE_DIM + ROPE_DIM  # 192
S = 2048
B = 1
NCORES = 8
HPC = N_HEAD // NCORES  # heads per core = 2
KSLICE = HIDDEN // NCORES  # 896
DOWN_COLS = Q_LORA + ROPE_DIM + KV_LORA  # 2112
INV_SQRT = 1.0 / math.sqrt(QK_DIM)
NEG = -1.0e30

_cache = {}


def _build_launch_a():
    nc = bacc.Bacc(None)
    h_in = nc.dram_tensor("h_in", [128, 7 * S], BF16, kind="ExternalInput")
    w_in = nc.dram_tensor("w_in", [128, 7 * DOWN_COLS], BF16, kind="ExternalInput")
    part = nc.dram_tensor("part", [S, DOWN_COLS], F32, kind="ExternalOutput")

    JT = [(0, 512), (512, 512), (1024, 512), (1536, 512), (2048, 64)]
    with TileContext(nc) as tc:
        with (
            tc.tile_pool(name="inp", bufs=1) as inp,
            tc.tile_pool(name="orow", bufs=3) as op,
            tc.tile_pool(name="ps", bufs=4, space="PSUM") as pp,
        ):
            hS = inp.tile([128, 7 * S], BF16)
            wS = inp.tile([128, 7 * DOWN_COLS], BF16)
            nc.sync.dma_start(hS[:, :], h_in[:, :])
            nc.sync.dma_start(wS[:, :], w_in[:, :])
            for st in range(16):
                orow = op.tile([128, DOWN_COLS], F32)
                for j0, jw in JT:
                    ps = pp.tile([128, 512], F32)
                    for kt in range(7):
                        nc.tensor.matmul(
                            ps[:, :jw],
                            hS[:, kt * S + st * 128 : kt * S + (st + 1) * 128],
                            wS[:, kt * DOWN_COLS + j0 : kt * DOWN_COLS + j0 + jw],
                            start=(kt == 0),
                            stop=(kt == 6),
                        )
                    nc.vector.tensor_copy(orow[:, j0 : j0 + jw], ps[:, :jw])
                nc.sync.dma_start(part[st * 128 : (st + 1) * 128, :], orow[:, :])
    nc.finalize()
    return nc


def _build_launch_b():
    nc = bacc.Bacc(None)
    qn_in = nc.dram_tensor("qn_in", [128, 12 * S], BF16, kind="ExternalInput")
    kv_in = nc.dram_tensor("kv_in", [128, 4 * S], BF16, kind="ExternalInput")
    kr_in = nc.dram_tensor("kr_in", [64, S], BF16, kind="ExternalInput")
    cos_in = nc.dram_tensor("cos_in", [64, S], F32, kind="ExternalInput")
    sin_in = nc.dram_tensor("sin_in", [64, S], F32, kind="ExternalInput")
    wq_in = nc.dram_tensor("wq_in", [128, 12 * 2 * QK_DIM], BF16, kind="ExternalInput")
    wkv_in = nc.dram_tensor("wkv_in", [128, 4 * 512], BF16, kind="ExternalInput")
    wo_in = nc.dram_tensor("wo_in", [128, 2 * HIDDEN], BF16, kind="ExternalInput")
    mask_in = nc.dram_tensor("mask_in", [128, 4 * 512], F32, kind="ExternalInput")
    attn_o = nc.dram_tensor("attn_o", [2 * S, S], F32, kind="ExternalOutput")
    out_o = nc.dram_tensor("out_o", [S, HIDDEN], BF16, kind="ExternalOutput")

    with TileContext(nc) as tc:
        with (
            tc.tile_pool(name="w", bufs=1) as wp,
            tc.tile_pool(name="act", bufs=1) as ap,
            tc.tile_pool(name="mm512", bufs=2, space="PSUM") as mmp,
            tc.tile_pool(name="s512", bufs=2, space="PSUM") as sp,
            tc.tile_pool(name="ptp", bufs=2, space="PSUM") as ptp,
            tc.tile_pool(name="aop", bufs=2, space="PSUM") as aop,
        ):
            wqS = wp.tile([128, 12 * 384], BF16)
            wkvS = wp.tile([128, 4 * 512], BF16)
            woS = wp.tile([128, 2 * HIDDEN], BF16)
            maskS = wp.tile([128, 4 * 512], F32)
            krS = wp.tile([64, S], BF16)
            cosS = wp.tile([64, S], F32)
            sinS = wp.tile([64, S], F32)
            ident = wp.tile([128, 128], BF16)
            nc.sync.dma_start(wqS[:, :], wq_in[:, :])
            nc.sync.dma_start(wkvS[:, :], wkv_in[:, :])
            nc.sync.dma_start(woS[:, :], wo_in[:, :])
            nc.sync.dma_start(maskS[:, :], mask_in[:, :])
            nc.sync.dma_start(krS[:, :], kr_in[:, :])
            nc.sync.dma_start(cosS[:, :], cos_in[:, :])
            nc.sync.dma_start(sinS[:, :], sin_in[:, :])
            make_identity(nc, ident[:, :])

            # per-head activations, resident
            qnopeT = [ap.tile([128, S], BF16, tag=f"qn{h}", name=f"qnopeT{h}") for h in range(2)]
            qropeT = [ap.tile([64, S], BF16, tag=f"qr{h}", name=f"qropeT{h}") for h in range(2)]
            knopeT = [ap.tile([128, S], BF16, tag=f"kn{h}", name=f"knopeT{h}") for h in range(2)]
            vS = [ap.tile([128, 16 * 128], BF16, tag=f"v{h}", name=f"vS{h}") for h in range(2)]
            aoT = [ap.tile([128, S], BF16, tag=f"ao{h}", name=f"aoT{h}") for h in range(2)]

            # ---- stage 1: q up-projection (+ RoPE on q) ----
            with (
                tc.tile_pool(name="qin", bufs=1) as qp,
                tc.tile_pool(name="rtmp", bufs=4) as rp,
            ):
                qnS = qp.tile([128, 12 * S], BF16)
                nc.sync.dma_start(qnS[:, :], qn_in[:, :])
                for h in range(2):
                    for st in range(4):
                        s0 = st * 512
                        # nope part: [128, 512] psum
                        ps = mmp.tile([128, 512], F32)
                        for kt in range(12):
                            nc.tensor.matmul(
                                ps[:, :],
                                wqS[:, kt * 384 + h * 192 : kt * 384 + h * 192 + 128],
                                qnS[:, kt * S + s0 : kt * S + s0 + 512],
                                start=(kt == 0),
                                stop=(kt == 11),
                            )
                        if st % 2 == 0:
                            nc.vector.tensor_copy(qnopeT[h][:, s0 : s0 + 512], ps[:, :])
                        else:
                            nc.scalar.copy(qnopeT[h][:, s0 : s0 + 512], ps[:, :])
                        # rope part: [64, 512] psum, then rotate
                        pr = mmp.tile([128, 512], F32, tag="ps")
                        for kt in range(12):
                            nc.tensor.matmul(
                                pr[:64, :],
                                wqS[:, kt * 384 + h * 192 + 128 : kt * 384 + h * 192 + 192],
                                qnS[:, kt * S + s0 : kt * S + s0 + 512],
                                start=(kt == 0),
                                stop=(kt == 11),
                            )
                        tcos = rp.tile([64, 512], F32, tag="tcos")
                        trot = rp.tile([64, 512], F32, tag="trot")
                        nc.vector.tensor_mul(tcos[:, :], pr[:64, :], cosS[:, s0 : s0 + 512])
                        nc.vector.tensor_mul(trot[:32, :], pr[32:64, :], sinS[:32, s0 : s0 + 512])
                        nc.vector.tensor_mul(trot[32:64, :], pr[:32, :], sinS[32:64, s0 : s0 + 512])
                        nc.vector.tensor_sub(qropeT[h][:32, s0 : s0 + 512], tcos[:32, :], trot[:32, :])
                        nc.vector.tensor_add(qropeT[h][32:64, s0 : s0 + 512], tcos[32:64, :], trot[32:64, :])

            # ---- stage 2: k_nope / V up-projection ----
            with tc.tile_pool(name="kvin", bufs=1) as kp:
                kvS = kp.tile([128, 4 * S], BF16)
                nc.sync.dma_start(kvS[:, :], kv_in[:, :])
                for h in range(2):
                    for st in range(4):
                        s0 = st * 512
                        ps = mmp.tile([128, 512], F32)
                        for kt in range(4):
                            nc.tensor.matmul(
                                ps[:, :],
                                wkvS[:, kt * 512 + h * 256 : kt * 512 + h * 256 + 128],
                                kvS[:, kt * S + s0 : kt * S + s0 + 512],
                                start=(kt == 0),
                                stop=(kt == 3),
                            )
                        nc.vector.tensor_copy(knopeT[h][:, s0 : s0 + 512], ps[:, :])
                    for ks in range(16):
                        pv = aop.tile([128, 128], F32, tag="pa")
                        for kt in range(4):
                            nc.tensor.matmul(
                                pv[:, :],
                                kvS[:, kt * S + ks * 128 : kt * S + (ks + 1) * 128],
                                wkvS[:, kt * 512 + h * 256 + 128 : kt * 512 + h * 256 + 256],
                                start=(kt == 0),
                                stop=(kt == 3),
                            )
                        nc.vector.tensor_copy(vS[h][:, ks * 128 : (ks + 1) * 128], pv[:, :])

            # ---- stage 3: attention per head ----
            with (
                tc.tile_pool(name="srow", bufs=2) as srp,
                tc.tile_pool(name="p32", bufs=2) as p32p,
                tc.tile_pool(name="af", bufs=2) as afp,
                tc.tile_pool(name="pb", bufs=2) as pbp,
                tc.tile_pool(name="pt", bufs=3) as pts,
                tc.tile_pool(name="stat", bufs=8) as stp,
            ):
                for qt in range(16):
                    for h in range(2):
                        dt = qt // 4
                        kend = 512 * (dt + 1)
                        q0 = qt * 128
                        srow = srp.tile([128, S], F32)
                        for kt in range(dt + 1):
                            k0 = kt * 512
                            ps = sp.tile([128, 512], F32)
                            nc.tensor.matmul(
                                ps[:, :],
                                qnopeT[h][:, q0 : q0 + 128],
                                knopeT[h][:, k0 : k0 + 512],
                                start=True,
                                stop=False,
                            )
                            nc.tensor.matmul(
                                ps[:, :],
                                qropeT[h][:, q0 : q0 + 128],
                                krS[:, k0 : k0 + 512],
                                start=False,
                                stop=True,
                            )
                            if kt == dt:
                                m = qt % 4
                                nc.vector.tensor_add(
                                    srow[:, k0 : k0 + 512],
                                    ps[:, :],
                                    maskS[:, m * 512 : (m + 1) * 512],
                                )
                            else:
                                nc.vector.tensor_copy(srow[:, k0 : k0 + 512], ps[:, :])
                        mx = stp.tile([128, 1], F32, tag="mx")
                        nb = stp.tile([128, 1], F32, tag="nb")
                        sm = stp.tile([128, 1], F32, tag="sm")
                        rr = stp.tile([128, 1], F32, tag="rr")
                        nc.vector.reduce_max(mx[:, :], srow[:, :kend], axis=mybir.AxisListType.X)
                        nc.vector.tensor_scalar_mul(nb[:, :], mx[:, :], -INV_SQRT)
                        p32 = p32p.tile([128, S], F32)
                        nc.scalar.activation(
                            p32[:, :kend],
                            srow[:, :kend],
                            mybir.ActivationFunctionType.Exp,
                            bias=nb[:, :],
                            scale=INV_SQRT,
                            accum_out=sm[:, :],
                        )
                        nc.vector.reciprocal(rr[:, :], sm[:, :])
                        af = afp.tile([128, S], F32)
                        nc.vector.tensor_scalar_mul(af[:, :kend], p32[:, :kend], rr[:, :])
                        nc.sync.dma_start(
                            attn_o[h * S + q0 : h * S + q0 + 128, 0:kend], af[:, :kend]
                        )
                        pb = pbp.tile([128, S], BF16)
                        nc.scalar.mul(pb[:, :kend], p32[:, :kend], rr[:, :])
                        # transpose P and accumulate A @ V (transposed): aoT = V^T P^T
                        pa = aop.tile([128, 128], F32, tag="pa")
                        nblk = qt + 1
                        for g in range((nblk + 3) // 4):
                            gw = min(4, nblk - 4 * g)
                            pt_ps = ptp.tile([128, 512], BF16)
                            for i in range(gw):
                                kb = 4 * g + i
                                nc.tensor.transpose(
                                    pt_ps[:, i * 128 : (i + 1) * 128],
                                    pb[:, kb * 128 : (kb + 1) * 128],
                                    ident[:, :],
                                )
                            ptile = pts.tile([128, 512], BF16)
                            nc.scalar.copy(ptile[:, : gw * 128], pt_ps[:, : gw * 128])
                            for i in range(gw):
                                kb = 4 * g + i
                                nc.tensor.matmul(
                                    pa[:, :],
                                    vS[h][:, kb * 128 : (kb + 1) * 128],
                                    ptile[:, i * 128 : (i + 1) * 128],
                                    start=(kb == 0),
                                    stop=(kb == qt),
                                )
                        nc.vector.tensor_copy(aoT[h][:, q0 : q0 + 128], pa[:, :])

            # ---- stage 4: output projection (partial over this core's heads) ----
            with tc.tile_pool(name="orow", bufs=2) as orp:
                for st in range(16):
                    s0 = st * 128
                    orow = orp.tile([128, HIDDEN], BF16)
                    for jt in range(14):
                        j0 = jt * 512
                        ps = mmp.tile([128, 512], F32)
                        nc.tensor.matmul(
                            ps[:, :],
                            aoT[0][:, s0 : s0 + 128],
                            woS[:, j0 : j0 + 512],
                            start=True,
                            stop=False,
                        )
                        nc.tensor.matmul(
                            ps[:, :],
                            aoT[1][:, s0 : s0 + 128],
                            woS[:, HIDDEN + j0 : HIDDEN + j0 + 512],
                            start=False,
                            stop=True,
                        )
                        if jt % 2 == 0:
                            nc.vector.tensor_copy(orow[:, j0 : j0 + 512], ps[:, :])
                        else:
                            nc.scalar.copy(orow[:, j0 : j0 + 512], ps[:, :])
                    nc.sync.dma_start(out_o[s0 : s0 + 128, :], orow[:, :])
    nc.finalize()
    return nc


def _get_progs():
    if "A" not in _cache:
        _cache["A"] = _build_launch_a()
        _cache["B"] = _build_launch_b()
    return _cache["A"], _cache["B"]


def _pack_p_major(arr, nt, width):
    # [nt*128, width] -> [128, nt*width] with [p, t*width + j] = arr[t*128+p, j]
    return np.ascontiguousarray(
        arr.reshape(nt, 128, width).swapaxes(0, 1).reshape(128, nt * width)
    )


def _to_bf16(a):
    return np.ascontiguousarray(a.astype(ml_dtypes.bfloat16))


def kernel(hidden_states, position_ids, Wq_down, q_norm_w, Wq_up,
           Wkv_down, kv_norm_w, Wkv_up, Wout, _trace=False):
    nc_a, nc_b = _get_progs()
    h = np.asarray(hidden_states, np.float32)[0]  # [S, HIDDEN]
    pos = np.asarray(position_ids).reshape(-1).astype(np.int64)

    # ---- launch A: down projections, contraction-sharded ----
    hT = _to_bf16(h.T)  # [HIDDEN, S]
    Wd = _to_bf16(np.concatenate([np.asarray(Wq_down, np.float32),
                                  np.asarray(Wkv_down, np.float32)], axis=1))
    in_maps_a = []
    for c in range(NCORES):
        r0 = c * KSLICE
        in_maps_a.append({
            "h_in": _pack_p_major(hT[r0 : r0 + KSLICE], 7, S),
            "w_in": _pack_p_major(Wd[r0 : r0 + KSLICE], 7, DOWN_COLS),
        })
    res_a = run_bass_kernel_spmd(nc_a, in_maps_a, core_ids=list(range(NCORES)),
                                 trace=_trace)
    acc = np.zeros((S, DOWN_COLS), np.float32)
    for c in range(NCORES):
        acc += res_a.results[c]["part"]

    qd = acc[:, :Q_LORA]
    kr_raw = acc[:, Q_LORA : Q_LORA + ROPE_DIM]
    kvh = acc[:, Q_LORA + ROPE_DIM :]

    # ---- host glue: RMSNorm, RoPE tables, k RoPE, repacks ----
    qrms = np.sqrt(np.mean(qd * qd, axis=-1, keepdims=True))
    qn = np.asarray(q_norm_w, np.float32) * (qd / (qrms + EPS))
    krms = np.sqrt(np.mean(kvh * kvh, axis=-1, keepdims=True))
    kvn = np.asarray(kv_norm_w, np.float32) * (kvh / (krms + EPS))

    inv_freq = 1.0 / (ROPE_BASE ** (np.arange(0, ROPE_DIM, 2, dtype=np.float32) / ROPE_DIM))
    t = np.arange(S, dtype=np.float32)
    freqs = np.outer(t, inv_freq)
    emb = np.concatenate([freqs, freqs], axis=-1)  # [S, 64]
    cos = np.cos(emb)[pos]  # [S, 64]
    sin = np.sin(emb)[pos]

    # interleaved -> half reorder, then rope, for the shared k_rope head
    kr_p = np.empty_like(kr_raw)
    kr_p[:, : ROPE_DIM // 2] = kr_raw[:, 0::2]
    kr_p[:, ROPE_DIM // 2 :] = kr_raw[:, 1::2]
    rot = np.concatenate([-kr_p[:, ROPE_DIM // 2 :], kr_p[:, : ROPE_DIM // 2]], axis=1)
    kr = kr_p * cos + rot * sin

    qn_in = _pack_p_major(_to_bf16(qn.T), 12, S)
    kv_in = _pack_p_major(_to_bf16(kvn.T), 4, S)
    kr_in = _to_bf16(kr.T)
    cos_in = np.ascontiguousarray(cos.T)
    sin_in = np.ascontiguousarray(sin.T)

    # weights with the q-rope interleave folded into Wq_up columns
    Wq_up_r = np.asarray(Wq_up, np.float32).reshape(Q_LORA, N_HEAD, QK_DIM)
    Wq_perm = Wq_up_r.copy()
    Wq_perm[:, :, NOPE_DIM : NOPE_DIM + ROPE_DIM // 2] = Wq_up_r[:, :, NOPE_DIM::2]
    Wq_perm[:, :, NOPE_DIM + ROPE_DIM // 2 :] = Wq_up_r[:, :, NOPE_DIM + 1 :: 2]
    Wkv_up_r = np.asarray(Wkv_up, np.float32).reshape(KV_LORA, N_HEAD, NOPE_DIM + V_DIM)
    Wout_f = np.asarray(Wout, np.float32)

    mask = np.full((128, 4, 512), NEG, np.float32)
    for m in range(4):
        for p in range(128):
            mask[p, m, : 128 * m + p + 1] = 0.0
    mask_in = np.ascontiguousarray(mask.reshape(128, 4 * 512))

    in_maps_b = []
    for c in range(NCORES):
        wq_c = _to_bf16(Wq_perm[:, 2 * c : 2 * c + 2, :].reshape(Q_LORA, 2 * QK_DIM))
        wkv_c = _to_bf16(Wkv_up_r[:, 2 * c : 2 * c + 2, :].reshape(KV_LORA, 512))
        wo_c = _to_bf16(Wout_f[256 * c : 256 * (c + 1)])
        in_maps_b.append({
            "qn_in": qn_in,
            "kv_in": kv_in,
            "kr_in": kr_in,
            "cos_in": cos_in,
            "sin_in": sin_in,
            "wq_in": _pack_p_major(wq_c, 12, 2 * QK_DIM),
            "wkv_in": _pack_p_major(wkv_c, 4, 512),
            "wo_in": _pack_p_major(wo_c, 2, HIDDEN),
            "mask_in": mask_in,
        })
    res_b = run_bass_kernel_spmd(nc_b, in_maps_b, core_ids=list(range(NCORES)),
                                 trace=_trace)

    attn = np.empty((1, N_HEAD, S, S), np.float32)
    out = np.zeros((S, HIDDEN), np.float32)
    for c in range(NCORES):
        attn[0, 2 * c : 2 * c + 2] = res_b.results[c]["attn_o"].reshape(2, S, S)
        out += res_b.results[c]["out_o"].astype(np.float32)

    if _trace:
        kernel._last_exec_ns = (res_a.exec_time_ns or 0) + (res_b.exec_time_ns or 0)
        kernel._last_parts = (res_a.exec_time_ns, res_b.exec_time_ns)
    return out.reshape(B, S, HIDDEN), attn


# revision 10
# speedup vs baseline: 1.0771x; 1.0170x over previous
"""MLA forward (DeepSeek-style) on 8 Trainium2 NeuronCores via Bass/Tile.

Two SPMD launches:
  A) down-projections, contraction-sharded over HIDDEN (8 x 896):
     each core computes a partial sum of [S, Q_LORA + ROPE + KV_LORA];
     host reduces the 8 partials.
  B) head-sharded (2 heads/core): RMSNorm'd low-rank inputs are
     up-projected, RoPE'd, attention + softmax run per head, attention
     probs written out, and the output projection accumulated over the
     core's 2 heads (host sums the 8 partial output projections).

All matmuls run in bf16 with fp32 PSUM accumulation; softmax is fp32.
"""

import sys

sys.path.insert(0, "/opt/trn_rl_repo")

import math

import numpy as np
import ml_dtypes

import concourse.bass as bass
import concourse.bacc as bacc
import concourse.mybir as mybir
from concourse.tile import TileContext
from concourse.masks import make_identity
from concourse.bass_utils import run_bass_kernel_spmd

BF16 = mybir.dt.bfloat16
F32 = mybir.dt.float32

HIDDEN = 7168
N_HEAD = 16
Q_LORA = 1536
KV_LORA = 512
ROPE_DIM = 64
NOPE_DIM = 128
V_DIM = 128
ROPE_BASE = 128000.0
EPS = 1e-6
QK_DIM = NOPE_DIM + ROPE_DIM  # 192
S = 2048
B = 1
NCORES = 8
HPC = N_HEAD // NCORES  # heads per core = 2
KSLICE = HIDDEN // NCORES  # 896
DOWN_COLS = Q_LORA + ROPE_DIM + KV_LORA  # 2112
INV_SQRT = 1.0 / math.sqrt(QK_DIM)
NEG = -1.0e30

_cache = {}


def _build_launch_a():
    nc = bacc.Bacc(None)
    h_in = nc.dram_tensor("h_in", [128, 7 * S], BF16, kind="ExternalInput")
    w_in = nc.dram_tensor("w_in", [128, 7 * DOWN_COLS], BF16, kind="ExternalInput")
    part = nc.dram_tensor("part", [S, DOWN_COLS], F32, kind="ExternalOutput")

    JT = [(0, 512), (512, 512), (1024, 512), (1536, 512), (2048, 64)]
    with TileContext(nc) as tc:
        with (
            tc.tile_pool(name="inp", bufs=1) as inp,
            tc.tile_pool(name="orow", bufs=3) as op,
            tc.tile_pool(name="ps", bufs=4, space="PSUM") as pp,
        ):
            hS = inp.tile([128, 7 * S], BF16)
            wS = inp.tile([128, 7 * DOWN_COLS], BF16)
            nc.sync.dma_start(hS[:, :], h_in[:, :])
            nc.sync.dma_start(wS[:, :], w_in[:, :])
            for st in range(16):
                orow = op.tile([128, DOWN_COLS], F32)
                for j0, jw in JT:
                    ps = pp.tile([128, 512], F32)
                    for kt in range(7):
                        nc.tensor.matmul(
                            ps[:, :jw],
                            hS[:, kt * S + st * 128 : kt * S + (st + 1) * 128],
                            wS[:, kt * DOWN_COLS + j0 : kt * DOWN_COLS + j0 + jw],
                            start=(kt == 0),
                            stop=(kt == 6),
                        )
                    nc.vector.tensor_copy(orow[:, j0 : j0 + jw], ps[:, :jw])
                nc.sync.dma_start(part[st * 128 : (st + 1) * 128, :], orow[:, :])
    nc.finalize()
    return nc


def _build_launch_b():
    nc = bacc.Bacc(None)
    qn_in = nc.dram_tensor("qn_in", [128, 12 * S], BF16, kind="ExternalInput")
    kv_in = nc.dram_tensor("kv_in", [128, 4 * S], BF16, kind="ExternalInput")
    kr_in = nc.dram_tensor("kr_in", [64, S], BF16, kind="ExternalInput")
    cos_in = nc.dram_tensor("cos_in", [64, S], F32, kind="ExternalInput")
    sin_in = nc.dram_tensor("sin_in", [64, S], F32, kind="ExternalInput")
    wq_in = nc.dram_tensor("wq_in", [128, 12 * 2 * QK_DIM], BF16, kind="ExternalInput")
    wkv_in = nc.dram_tensor("wkv_in", [128, 4 * 512], BF16, kind="ExternalInput")
    wo_in = nc.dram_tensor("wo_in", [128, 2 * HIDDEN], BF16, kind="ExternalInput")
    mask_in = nc.dram_tensor("mask_in", [128, 4 * 512], F32, kind="ExternalInput")
    attn_o = nc.dram_tensor("attn_o", [2 * S, S], F32, kind="ExternalOutput")
    out_o = nc.dram_tensor("out_o", [S, HIDDEN], BF16, kind="ExternalOutput")

    with TileContext(nc) as tc:
        with (
            tc.tile_pool(name="w", bufs=1) as wp,
            tc.tile_pool(name="act", bufs=1) as ap,
            tc.tile_pool(name="mm512", bufs=2, space="PSUM") as mmp,
            tc.tile_pool(name="s512", bufs=2, space="PSUM") as sp,
            tc.tile_pool(name="ptp", bufs=2, space="PSUM") as ptp,
            tc.tile_pool(name="aop", bufs=2, space="PSUM") as aop,
        ):
            wqS = wp.tile([128, 12 * 384], BF16)
            wkvS = wp.tile([128, 4 * 512], BF16)
            woS = wp.tile([128, 2 * HIDDEN], BF16)
            maskS = wp.tile([128, 4 * 512], F32)
            krS = wp.tile([64, S], BF16)
            cosS = wp.tile([64, S], F32)
            sinS = wp.tile([64, S], F32)
            ident = wp.tile([128, 128], BF16)
            nc.sync.dma_start(wqS[:, :], wq_in[:, :])
            nc.sync.dma_start(wkvS[:, :], wkv_in[:, :])
            nc.sync.dma_start(woS[:, :], wo_in[:, :])
            nc.sync.dma_start(maskS[:, :], mask_in[:, :])
            nc.sync.dma_start(krS[:, :], kr_in[:, :])
            nc.sync.dma_start(cosS[:, :], cos_in[:, :])
            nc.sync.dma_start(sinS[:, :], sin_in[:, :])
            make_identity(nc, ident[:, :])

            # per-head activations, resident
            qnopeT = [ap.tile([128, S], BF16, tag=f"qn{h}", name=f"qnopeT{h}") for h in range(2)]
            qropeT = [ap.tile([64, S], BF16, tag=f"qr{h}", name=f"qropeT{h}") for h in range(2)]
            knopeT = [ap.tile([128, S], BF16, tag=f"kn{h}", name=f"knopeT{h}") for h in range(2)]
            vS = [ap.tile([128, 16 * 128], BF16, tag=f"v{h}", name=f"vS{h}") for h in range(2)]
            aoT = [ap.tile([128, S], BF16, tag=f"ao{h}", name=f"aoT{h}") for h in range(2)]

            # ---- stage 1: q up-projection (+ RoPE on q) ----
            with (
                tc.tile_pool(name="qin", bufs=1) as qp,
                tc.tile_pool(name="rtmp", bufs=4) as rp,
            ):
                qnS = qp.tile([128, 12 * S], BF16)
                nc.sync.dma_start(qnS[:, :], qn_in[:, :])
                for h in range(2):
                    for st in range(4):
                        s0 = st * 512
                        # nope part: [128, 512] psum
                        ps = mmp.tile([128, 512], F32)
                        for kt in range(12):
                            nc.tensor.matmul(
                                ps[:, :],
                                wqS[:, kt * 384 + h * 192 : kt * 384 + h * 192 + 128],
                                qnS[:, kt * S + s0 : kt * S + s0 + 512],
                                start=(kt == 0),
                                stop=(kt == 11),
                            )
                        if st % 2 == 0:
                            nc.vector.tensor_copy(qnopeT[h][:, s0 : s0 + 512], ps[:, :])
                        else:
                            nc.scalar.copy(qnopeT[h][:, s0 : s0 + 512], ps[:, :])
                        # rope part: [64, 512] psum, then rotate
                        pr = mmp.tile([128, 512], F32, tag="ps")
                        for kt in range(12):
                            nc.tensor.matmul(
                                pr[:64, :],
                                wqS[:, kt * 384 + h * 192 + 128 : kt * 384 + h * 192 + 192],
                                qnS[:, kt * S + s0 : kt * S + s0 + 512],
                                start=(kt == 0),
                                stop=(kt == 11),
                            )
                        tcos = rp.tile([64, 512], F32, tag="tcos")
                        trot = rp.tile([64, 512], F32, tag="trot")
                        nc.vector.tensor_mul(tcos[:, :], pr[:64, :], cosS[:, s0 : s0 + 512])
                        nc.vector.tensor_mul(trot[:32, :], pr[32:64, :], sinS[:32, s0 : s0 + 512])
                        nc.vector.tensor_mul(trot[32:64, :], pr[:32, :], sinS[32:64, s0 : s0 + 512])
                        nc.vector.tensor_sub(qropeT[h][:32, s0 : s0 + 512], tcos[:32, :], trot[:32, :])
                        nc.vector.tensor_add(qropeT[h][32:64, s0 : s0 + 512], tcos[32:64, :], trot[32:64, :])

            # ---- stage 2: k_nope / V up-projection ----
            with tc.tile_pool(name="kvin", bufs=1) as kp:
                kvS = kp.tile([128, 4 * S], BF16)
                nc.sync.dma_start(kvS[:, :], kv_in[:, :])
                for h in range(2):
                    for st in range(4):
                        s0 = st * 512
                        ps = mmp.tile([128, 512], F32)
                        for kt in range(4):
                            nc.tensor.matmul(
                                ps[:, :],
                                wkvS[:, kt * 512 + h * 256 : kt * 512 + h * 256 + 128],
                                kvS[:, kt * S + s0 : kt * S + s0 + 512],
                                start=(kt == 0),
                                stop=(kt == 3),
                            )
                        nc.vector.tensor_copy(knopeT[h][:, s0 : s0 + 512], ps[:, :])
                    for ks in range(16):
                        pv = aop.tile([128, 128], F32, tag="pa")
                        for kt in range(4):
                            nc.tensor.matmul(
                                pv[:, :],
                                kvS[:, kt * S + ks * 128 : kt * S + (ks + 1) * 128],
                                wkvS[:, kt * 512 + h * 256 + 128 : kt * 512 + h * 256 + 256],
                                start=(kt == 0),
                                stop=(kt == 3),
                            )
                        nc.vector.tensor_copy(vS[h][:, ks * 128 : (ks + 1) * 128], pv[:, :])

            # ---- stage 3: attention per head ----
            with (
                tc.tile_pool(name="srow", bufs=3) as srp,
                tc.tile_pool(name="p32", bufs=3) as p32p,
                tc.tile_pool(name="af", bufs=3) as afp,
                tc.tile_pool(name="pb", bufs=3) as pbp,
                tc.tile_pool(name="pt", bufs=3) as pts,
                tc.tile_pool(name="stat", bufs=8) as stp,
            ):
                for qt in range(16):
                    for h in range(2):
                        dt = qt // 4
                        kend = 512 * (dt + 1)
                        q0 = qt * 128
                        srow = srp.tile([128, S], F32)
                        for kt in range(dt + 1):
                            k0 = kt * 512
                            ps = (sp if (2 * qt + h + kt) % 2 == 0 else mmp).tile(
                                [128, 512], F32, tag="ps", name="ps"
                            )
                            nc.tensor.matmul(
                                ps[:, :],
                                qnopeT[h][:, q0 : q0 + 128],
                                knopeT[h][:, k0 : k0 + 512],
                                start=True,
                                stop=False,
                            )
                            nc.tensor.matmul(
                                ps[:, :],
                                qropeT[h][:, q0 : q0 + 128],
                                krS[:, k0 : k0 + 512],
                                start=False,
                                stop=True,
                            )
                            if kt == dt:
                                m = qt % 4
                                nc.vector.tensor_add(
                                    srow[:, k0 : k0 + 512],
                                    ps[:, :],
                                    maskS[:, m * 512 : (m + 1) * 512],
                                )
                            else:
                                nc.vector.tensor_copy(srow[:, k0 : k0 + 512], ps[:, :])
                        mx = stp.tile([128, 1], F32, tag="mx")
                        nb = stp.tile([128, 1], F32, tag="nb")
                        sm = stp.tile([128, 1], F32, tag="sm")
                        rr = stp.tile([128, 1], F32, tag="rr")
                        nc.vector.reduce_max(mx[:, :], srow[:, :kend], axis=mybir.AxisListType.X)
                        nc.vector.tensor_scalar_mul(nb[:, :], mx[:, :], -INV_SQRT)
                        p32 = p32p.tile([128, S], F32)
                        nc.scalar.activation(
                            p32[:, :kend],
                            srow[:, :kend],
                            mybir.ActivationFunctionType.Exp,
                            bias=nb[:, :],
                            scale=INV_SQRT,
                            accum_out=sm[:, :],
                        )
                        nc.vector.reciprocal(rr[:, :], sm[:, :])
                        af = afp.tile([128, S], F32)
                        nc.vector.tensor_scalar_mul(af[:, :kend], p32[:, :kend], rr[:, :])
                        nc.sync.dma_start(
                            attn_o[h * S + q0 : h * S + q0 + 128, 0:kend], af[:, :kend]
                        )
                        pb = pbp.tile([128, S], BF16)
                        nc.scalar.mul(pb[:, :kend], p32[:, :kend], rr[:, :])
                        # transpose P and accumulate A @ V (transposed): aoT = V^T P^T
                        pa = aop.tile([128, 128], F32, tag="pa")
                        nblk = qt + 1
                        for g in range((nblk + 3) // 4):
                            gw = min(4, nblk - 4 * g)
                            pt_ps = ptp.tile([128, 512], BF16)
                            for i in range(gw):
                                kb = 4 * g + i
                                nc.tensor.transpose(
                                    pt_ps[:, i * 128 : (i + 1) * 128],
                                    pb[:, kb * 128 : (kb + 1) * 128],
                                    ident[:, :],
                                )
                            ptile = pts.tile([128, 512], BF16)
                            nc.scalar.copy(ptile[:, : gw * 128], pt_ps[:, : gw * 128])
                            for i in range(gw):
                                kb = 4 * g + i
                                nc.tensor.matmul(
                                    pa[:, :],
                                    vS[h][:, kb * 128 : (kb + 1) * 128],
                                    ptile[:, i * 128 : (i + 1) * 128],
                                    start=(kb == 0),
                                    stop=(kb == qt),
                                )
                        nc.vector.tensor_copy(aoT[h][:, q0 : q0 + 128], pa[:, :])

            # ---- stage 4: output projection (partial over this core's heads) ----
            with tc.tile_pool(name="orow", bufs=2) as orp:
                for st in range(16):
                    s0 = st * 128
                    orow = orp.tile([128, HIDDEN], BF16)
                    for jt in range(14):
                        j0 = jt * 512
                        ps = mmp.tile([128, 512], F32)
                        nc.tensor.matmul(
                            ps[:, :],
                            aoT[0][:, s0 : s0 + 128],
                            woS[:, j0 : j0 + 512],
                            start=True,
                            stop=False,
                        )
                        nc.tensor.matmul(
                            ps[:, :],
                            aoT[1][:, s0 : s0 + 128],
                            woS[:, HIDDEN + j0 : HIDDEN + j0 + 512],
                            start=False,
                            stop=True,
                        )
                        if jt % 2 == 0:
                            nc.vector.tensor_copy(orow[:, j0 : j0 + 512], ps[:, :])
                        else:
                            nc.scalar.copy(orow[:, j0 : j0 + 512], ps[:, :])
                    nc.sync.dma_start(out_o[s0 : s0 + 128, :], orow[:, :])
    nc.finalize()
    return nc


def _get_progs():
    if "A" not in _cache:
        _cache["A"] = _build_launch_a()
        _cache["B"] = _build_launch_b()
    return _cache["A"], _cache["B"]


def _pack_p_major(arr, nt, width):
    # [nt*128, width] -> [128, nt*width] with [p, t*width + j] = arr[t*128+p, j]
    return np.ascontiguousarray(
        arr.reshape(nt, 128, width).swapaxes(0, 1).reshape(128, nt * width)
    )


def _to_bf16(a):
    return np.ascontiguousarray(a.astype(ml_dtypes.bfloat16))


def kernel(hidden_states, position_ids, Wq_down, q_norm_w, Wq_up,
           Wkv_down, kv_norm_w, Wkv_up, Wout, _trace=False):
    nc_a, nc_b = _get_progs()
    h = np.asarray(hidden_states, np.float32)[0]  # [S, HIDDEN]
    pos = np.asarray(position_ids).reshape(-1).astype(np.int64)

    # ---- launch A: down projections, contraction-sharded ----
    hT = _to_bf16(h.T)  # [HIDDEN, S]
    Wd = _to_bf16(np.concatenate([np.asarray(Wq_down, np.float32),
                                  np.asarray(Wkv_down, np.float32)], axis=1))
    in_maps_a = []
    for c in range(NCORES):
        r0 = c * KSLICE
        in_maps_a.append({
            "h_in": _pack_p_major(hT[r0 : r0 + KSLICE], 7, S),
            "w_in": _pack_p_major(Wd[r0 : r0 + KSLICE], 7, DOWN_COLS),
        })
    res_a = run_bass_kernel_spmd(nc_a, in_maps_a, core_ids=list(range(NCORES)),
                                 trace=_trace)
    acc = np.zeros((S, DOWN_COLS), np.float32)
    for c in range(NCORES):
        acc += res_a.results[c]["part"]

    qd = acc[:, :Q_LORA]
    kr_raw = acc[:, Q_LORA : Q_LORA + ROPE_DIM]
    kvh = acc[:, Q_LORA + ROPE_DIM :]

    # ---- host glue: RMSNorm, RoPE tables, k RoPE, repacks ----
    qrms = np.sqrt(np.mean(qd * qd, axis=-1, keepdims=True))
    qn = np.asarray(q_norm_w, np.float32) * (qd / (qrms + EPS))
    krms = np.sqrt(np.mean(kvh * kvh, axis=-1, keepdims=True))
    kvn = np.asarray(kv_norm_w, np.float32) * (kvh / (krms + EPS))

    inv_freq = 1.0 / (ROPE_BASE ** (np.arange(0, ROPE_DIM, 2, dtype=np.float32) / ROPE_DIM))
    t = np.arange(S, dtype=np.float32)
    freqs = np.outer(t, inv_freq)
    emb = np.concatenate([freqs, freqs], axis=-1)  # [S, 64]
    cos = np.cos(emb)[pos]  # [S, 64]
    sin = np.sin(emb)[pos]

    # interleaved -> half reorder, then rope, for the shared k_rope head
    kr_p = np.empty_like(kr_raw)
    kr_p[:, : ROPE_DIM // 2] = kr_raw[:, 0::2]
    kr_p[:, ROPE_DIM // 2 :] = kr_raw[:, 1::2]
    rot = np.concatenate([-kr_p[:, ROPE_DIM // 2 :], kr_p[:, : ROPE_DIM // 2]], axis=1)
    kr = kr_p * cos + rot * sin

    qn_in = _pack_p_major(_to_bf16(qn.T), 12, S)
    kv_in = _pack_p_major(_to_bf16(kvn.T), 4, S)
    kr_in = _to_bf16(kr.T)
    cos_in = np.ascontiguousarray(cos.T)
    sin_in = np.ascontiguousarray(sin.T)

    # weights with the q-rope interleave folded into Wq_up columns
    Wq_up_r = np.asarray(Wq_up, np.float32).reshape(Q_LORA, N_HEAD, QK_DIM)
    Wq_perm = Wq_up_r.copy()
    Wq_perm[:, :, NOPE_DIM : NOPE_DIM + ROPE_DIM // 2] = Wq_up_r[:, :, NOPE_DIM::2]
    Wq_perm[:, :, NOPE_DIM + ROPE_DIM // 2 :] = Wq_up_r[:, :, NOPE_DIM + 1 :: 2]
    Wkv_up_r = np.asarray(Wkv_up, np.float32).reshape(KV_LORA, N_HEAD, NOPE_DIM + V_DIM)
    Wout_f = np.asarray(Wout, np.float32)

    mask = np.full((128, 4, 512), NEG, np.float32)
    for m in range(4):
        for p in range(128):
            mask[p, m, : 128 * m + p + 1] = 0.0
    mask_in = np.ascontiguousarray(mask.reshape(128, 4 * 512))

    in_maps_b = []
    for c in range(NCORES):
        wq_c = _to_bf16(Wq_perm[:, 2 * c : 2 * c + 2, :].reshape(Q_LORA, 2 * QK_DIM))
        wkv_c = _to_bf16(Wkv_up_r[:, 2 * c : 2 * c + 2, :].reshape(KV_LORA, 512))
        wo_c = _to_bf16(Wout_f[256 * c : 256 * (c + 1)])
        in_maps_b.append({
            "qn_in": qn_in,
            "kv_in": kv_in,
            "kr_in": kr_in,
            "cos_in": cos_in,
            "sin_in": sin_in,
            "wq_in": _pack_p_major(wq_c, 12, 2 * QK_DIM),
            "wkv_in": _pack_p_major(wkv_c, 4, 512),
            "wo_in": _pack_p_major(wo_c, 2, HIDDEN),
            "mask_in": mask_in,
        })
    res_b = run_bass_kernel_spmd(nc_b, in_maps_b, core_ids=list(range(NCORES)),
                                 trace=_trace)

    attn = np.empty((1, N_HEAD, S, S), np.float32)
    out = np.zeros((S, HIDDEN), np.float32)
    for c in range(NCORES):
        attn[0, 2 * c : 2 * c + 2] = res_b.results[c]["attn_o"].reshape(2, S, S)
        out += res_b.results[c]["out_o"].astype(np.float32)

    if _trace:
        kernel._last_exec_ns = (res_a.exec_time_ns or 0) + (res_b.exec_time_ns or 0)
        kernel._last_parts = (res_a.exec_time_ns, res_b.exec_time_ns)
    return out.reshape(B, S, HIDDEN), attn
